# revision 20
# baseline (speedup 1.0000x reference)
"""BarCachedCrossAttention Trainium2 kernel.

Sharding: 8 cores = 4 batches x 2 head-groups (8 heads / 512 channels each).
Per core, everything is computed in a transposed layout (partition = context
token for scores) so probs never need a transpose: U^T = V'^T @ P^T with a
ones-column in V' producing the softmax denominators for free.  The
instrument mask is applied by zeroing masked tokens' V' rows (including the
ones-column), so exp needs no per-token bias and batches into 1024-wide ACT
ops.  The K/V projection and the attention (scores -> exp -> U accumulation)
are fused per context slab so ACT exp overlaps projection matmuls.

Host-side folds (pure input prep, the heavy GEMMs all run on device):
  - instrument/bar embeddings are added into the context once on the host
    (the same gather the reference does), so no combo-table / one-hot
    correction matmuls are needed on device;
  - K-bias is dropped (exactly cancels in softmax over n);
  - Q-bias + current instrument embedding fold into a per-channel bias
    applied by the Q-projection's PSUM->SBUF activation;
  - V-bias passes through softmax unchanged (weights sum to 1) and folds
    with the output bias: bo_eff = bo + bv @ Wo.T.

fp16 operands everywhere (1 cyc/row matmuls, FWL weight loads, half DMA);
PSUM/U stay f32.  exp uses a constant -12 shift (cancels in U/Z) keeping the
probs near unity.  Probs and V' are bf16 (range-safe under exp).  Score
matmul head-pairs share one PSUM tile so both 64-row groups become ready
together and run concurrently on the PE (row-group tiling).  1/Z runs at
bf16 for range (Z spans e^-9..2e6).
"""

import sys

sys.path.insert(0, "/opt/trn_rl_repo")

import numpy as np

import concourse.bacc as bacc
import concourse.tile as tile
from concourse import mybir
from concourse.bass_utils import run_bass_kernel_spmd

B, T, N_CTX, H = 4, 512, 2048, 1024
NUM_HEADS, NUM_INSTRUMENTS, MAX_BARS = 16, 16, 8
HEAD_DIM = H // NUM_HEADS  # 64
HG = 2  # head groups (cores per batch)
CH = H // HG  # 512 channels per core
NH_G = NUM_HEADS // HG  # 8 heads per core
P = 128
F32 = mybir.dt.float32
BF16 = mybir.dt.bfloat16
DT = mybir.dt.float16
SHIFT = -12.0  # exp shift keeps bf16 probs in a comfortable range

KC = H // P  # 8 contraction chunks for projections
PT_CH = CH // P  # 4 partition tiles of channels
NT = N_CTX // P  # 16 context tiles of 128 tokens
TT = T // P  # 4 tiles of query tokens
SLABS = [512, 512, 512, 256, 256]  # context slab sizes (sum = N_CTX)

_compiled = None


def _build():
    nc = bacc.Bacc("TRN2", target_bir_lowering=False, debug=False, num_devices=8)

    qT_d = nc.dram_tensor("qT", [P, KC, T], DT, kind="ExternalInput")
    ctxT_d = nc.dram_tensor("ctxT", [P, KC, N_CTX], DT, kind="ExternalInput")
    wq_d = nc.dram_tensor("wqT", [P, KC, CH], DT, kind="ExternalInput")
    wk_d = nc.dram_tensor("wkT", [P, KC, CH], DT, kind="ExternalInput")
    wv_d = nc.dram_tensor("wvT", [P, KC, CH], DT, kind="ExternalInput")
    wo_d = nc.dram_tensor("woT", [P, PT_CH, H], DT, kind="ExternalInput")
    mb_d = nc.dram_tensor("mb", [P, NT], F32, kind="ExternalInput")
    bqe_d = nc.dram_tensor("bqe", [P, PT_CH], F32, kind="ExternalInput")
    out_d = nc.dram_tensor("out", [T, H], F32, kind="ExternalOutput")

    with tile.TileContext(nc) as tc:
        with (
            nc.allow_low_precision(reason="fp16 matmul operands; accum stays f32"),
            tc.tile_pool(name="persist", bufs=1) as pers,
        ):
            # ---- input DMA in need order ----
            # sync: qt/wq chunk pairs (Q-proj streams behind them), small
            # tiles, then wk chunks (K-proj).  gpsimd: slab0 then slabs.
            # scalar (idle until first exp): wv, wo.
            qt = pers.tile([P, KC, T], DT, name="qt_in")
            wq = pers.tile([P, KC, CH], DT, name="wq")
            for k2 in range(0, KC, 2):
                nc.sync.dma_start(qt[:, k2 : k2 + 2, :], qT_d.ap()[:, k2 : k2 + 2, :])
                nc.sync.dma_start(wq[:, k2 : k2 + 2, :], wq_d.ap()[:, k2 : k2 + 2, :])
            mb = pers.tile([P, NT], F32, name="mb")
            nc.sync.dma_start(mb[:], mb_d.ap())
            bqe = pers.tile([P, PT_CH], F32, name="bqe")
            nc.sync.dma_start(bqe[:], bqe_d.ap())
            wk = pers.tile([P, KC, CH], DT, name="wk")
            wv = pers.tile([P, KC, CH], DT, name="wv")
            wo = pers.tile([P, PT_CH, H], DT, name="wo")

            ones1b = pers.tile([1, P], BF16, name="ones1b")
            nc.vector.memset(ones1b[:], 1.0)
            ones8 = pers.tile([P, NH_G], F32, name="ones8")
            nc.vector.memset(ones8[:], 1.0)
            shiftb = pers.tile([P, 1], F32, name="shiftb")
            nc.vector.memset(shiftb[:], SHIFT)

            QT = [pers.tile([P, T], DT, name=f"qt{p}") for p in range(PT_CH)]
            OT = [pers.tile([P, T], DT, name=f"ot{p}") for p in range(PT_CH)]
            U = [pers.tile([HEAD_DIM + 1, T], F32, name=f"u{h}") for h in range(NH_G)]
            ZS = [pers.tile([1, 2, 512], F32, name=f"zs{hp}") for hp in range(NH_G // 2)]
            RF = [pers.tile([1, 2, 512], F32, name=f"rf{hp}") for hp in range(NH_G // 2)]
            RT = [pers.tile([1, 2, 512], BF16, name=f"rt{hp}") for hp in range(NH_G // 2)]

            # ---- Q projection (k-major: streams behind the chunked DMA) ----
            with tc.tile_pool(name="qps", bufs=1, space="PSUM") as qps:
                ps_q = [qps.tile([P, 512], F32, name=f"ps_q{p}") for p in range(PT_CH)]
                for k in range(KC):
                    for p in range(PT_CH):
                        nc.tensor.matmul(
                            ps_q[p][:],
                            wq[:, k, p * P : (p + 1) * P],
                            qt[:, k, :],
                            start=(k == 0),
                            stop=(k == KC - 1),
                        )
                for p in range(PT_CH):
                    nc.scalar.activation(
                        QT[p][:], ps_q[p][:], mybir.ActivationFunctionType.Identity,
                        bias=bqe[:, p : p + 1], scale=1.0,
                    )

            # ---- fused K/V projection + attention, one context slab at a time ----
            NS = len(SLABS)
            offs = [sum(SLABS[:i]) for i in range(NS)]

            def emit_proj(si, kvsb, kvps, preloaded=None):
                """K^T and V' tiles for slab si; returns (kts, vts)."""
                n0, sl = offs[si], SLABS[si]
                s4n = sl // P
                if preloaded is None:
                    slab = slabp.tile([P, KC, 512], DT, name="slab")
                    nc.sync.dma_start(
                        slab[:, :, :sl], ctxT_d.ap()[:, :, n0 : n0 + sl]
                    )
                else:
                    slab = preloaded
                kts = []
                for p in range(PT_CH):
                    ps = kvps.tile([P, 512], F32, name="ps_kv")
                    for k in range(KC):
                        nc.tensor.matmul(
                            ps[:, :sl],
                            wk[:, k, p * P : (p + 1) * P],
                            slab[:, k, :sl],
                            start=(k == 0), stop=(k == KC - 1),
                        )
                    kt = kvsb.tile([P, 512], DT, name=f"kt{p}")
                    nc.vector.tensor_copy(kt[:, :sl], ps[:, :sl])
                    kts.append(kt)
                vts = []
                for s4 in range(s4n):
                    i = (n0 // P) + s4
                    psv = kvps.tile([P, 512], F32, name="ps_kv")
                    for k in range(KC):
                        nc.tensor.matmul(
                            psv[:],
                            slab[:, k, s4 * P : (s4 + 1) * P],
                            wv[:, k, :],
                            start=(k == 0), stop=(k == KC - 1),
                        )
                    vt = kvsb.tile([P, NH_G, HEAD_DIM + 1], BF16, name=f"v{s4}")
                    nc.vector.tensor_scalar_mul(
                        vt[:, :, :HEAD_DIM],
                        psv[:].rearrange("p (h d) -> p h d", d=HEAD_DIM),
                        mb[:, i : i + 1],
                    )
                    nc.vector.tensor_scalar_mul(
                        vt[:, :, HEAD_DIM], ones8[:], mb[:, i : i + 1]
                    )
                    vts.append(vt)
                return kts, vts

            with (
                tc.tile_pool(name="slab", bufs=2) as slabp,
                tc.tile_pool(name="kvsb", bufs=2) as kvsb,
                tc.tile_pool(name="ptp", bufs=4) as ptp,
                tc.tile_pool(name="kvps", bufs=2, space="PSUM") as kvps,
                tc.tile_pool(name="sps", bufs=2, space="PSUM") as sps,
                tc.tile_pool(name="ups", bufs=1, space="PSUM") as ups,
            ):
                def emit_norm(hp):
                    for hi in range(2):
                        h = 2 * hp + hi
                        psr = kvps.tile([P, 512], F32, name="ps_kv")
                        nc.tensor.matmul(
                            psr[:HEAD_DIM, :], ones1b[:, :HEAD_DIM],
                            RT[hp][:, hi, :], start=True, stop=True,
                        )
                        nc.vector.tensor_tensor(
                            OT[hp][hi * HEAD_DIM : (hi + 1) * HEAD_DIM, :],
                            U[h][:HEAD_DIM, :],
                            psr[:HEAD_DIM, :],
                            op=mybir.AluOpType.mult,
                        )

                # slab0 + wk interleaved chunk pairs, then wv chunks, then
                # wo -- all on the sync queue so the DMA engines see
                # descriptors in exact need order (big early transfers on
                # other queues would starve the critical path).
                first_slab = slabp.tile([P, KC, 512], DT, name="slab")
                for k2 in range(0, KC, 2):
                    nc.sync.dma_start(
                        first_slab[:, k2 : k2 + 2, :],
                        ctxT_d.ap()[:, k2 : k2 + 2, 0:512],
                    )
                    nc.sync.dma_start(
                        wk[:, k2 : k2 + 2, :], wk_d.ap()[:, k2 : k2 + 2, :]
                    )
                for k2 in range(0, KC, 2):
                    nc.sync.dma_start(
                        wv[:, k2 : k2 + 2, :], wv_d.ap()[:, k2 : k2 + 2, :]
                    )
                nc.sync.dma_start(wo[:], wo_d.ap())
                cur = emit_proj(0, kvsb, kvps, preloaded=first_slab)
                for ns in range(NS):
                    kts, vts = cur
                    s4n = SLABS[ns] // P
                    # attention: per (head pair, 128-token ctx chunk): the
                    # score pair shares one PSUM tile -> both row groups
                    # become ready together and run concurrently on the PE.
                    for hp in range(NH_G // 2):
                        psus = [
                            ups.tile([HEAD_DIM + 1, 512], F32, name=f"ps_u{hi}")
                            for hi in range(2)
                        ]
                        for s4 in range(s4n):
                            pss = sps.tile([P, 2, 512], F32, name="ps_s")
                            pts = ptp.tile([P, 2, 512], BF16, name="pt")
                            for hi in range(2):
                                d0, d1 = hi * HEAD_DIM, (hi + 1) * HEAD_DIM
                                nc.tensor.matmul(
                                    pss[:, hi, :],
                                    kts[hp][d0:d1, s4 * P : (s4 + 1) * P],
                                    QT[hp][d0:d1, :],
                                    start=True, stop=True,
                                )
                            nc.scalar.activation(
                                pts[:], pss[:], mybir.ActivationFunctionType.Exp,
                                bias=shiftb[:], scale=0.125,
                            )
                            for hi in range(2):
                                nc.tensor.matmul(
                                    psus[hi][:],
                                    vts[s4][:, 2 * hp + hi, :],
                                    pts[:, hi, :],
                                    start=(s4 == 0), stop=(s4 == s4n - 1),
                                )
                        for hi in range(2):
                            h = 2 * hp + hi
                            if ns == 0:
                                nc.vector.tensor_copy(U[h][:], psus[hi][:])
                            else:
                                nc.vector.tensor_add(U[h][:], U[h][:], psus[hi][:])
                        if ns == NS - 1:
                            # normalization, software-pipelined one head pair
                            # behind the attention: the DVE reciprocal chain
                            # for hp runs under hp+1's attention, and the psr
                            # broadcast + OT multiply for hp-1 are emitted
                            # here so OT is written well before the O
                            # projection reads it.
                            for hi in range(2):
                                h = 2 * hp + hi
                                nc.vector.tensor_copy(
                                    ZS[hp][:, hi, :],
                                    U[h][HEAD_DIM : HEAD_DIM + 1, :],
                                )
                            nc.vector.reciprocal_approx_fast(RF[hp][:], ZS[hp][:])
                            nc.vector.tensor_copy(RT[hp][:], RF[hp][:])
                            if hp > 0:
                                emit_norm(hp - 1)
                    if ns + 1 < NS:
                        cur = emit_proj(ns + 1, kvsb, kvps)
                    else:
                        emit_norm(NH_G // 2 - 1)

            # ---- normalization broadcast + output projection ----
            with (
                tc.tile_pool(name="ob", bufs=3) as obp,
                tc.tile_pool(name="ops", bufs=2, space="PSUM") as ops,
            ):
                # O = OT.T @ WoT (partial over this head-group's channels)
                for tt in range(TT):
                    for o in range(2):
                        pso = ops.tile([P, 512], F32, name="ps_o")
                        for p in range(PT_CH):
                            nc.tensor.matmul(
                                pso[:],
                                OT[p][:, tt * P : (tt + 1) * P],
                                wo[:, p, o * 512 : (o + 1) * 512],
                                start=(p == 0), stop=(p == PT_CH - 1),
                            )
                        ob = obp.tile([P, 512], F32, name="ob")
                        if o == 0:
                            nc.vector.tensor_copy(ob[:], pso[:])
                        else:
                            nc.scalar.copy(ob[:], pso[:])
                        nc.sync.dma_start(
                            out_d.ap()[tt * P : (tt + 1) * P, o * 512 : (o + 1) * 512],
                            ob[:],
                        )

    nc.compile()
    return nc


def _prep_inputs(query, context, instrument_ids, current_instrument_id, bar_offsets,
                 Wq, bq, Wk, bk, Wv, bv, Wo, bo, inst_emb, bar_emb):
    f32, f16 = np.float32, np.float16
    query = np.asarray(query, f32)
    context = np.asarray(context, f32)
    inst = np.asarray(instrument_ids).astype(np.int64)
    bars = np.clip(np.asarray(bar_offsets).astype(np.int64), 0, MAX_BARS - 1)
    cur = min(max(int(np.asarray(current_instrument_id)), 0), NUM_INSTRUMENTS - 1)
    Wq, Wk, Wv, Wo = (np.asarray(w, f32) for w in (Wq, Wk, Wv, Wo))
    bq, bv, bo = (np.asarray(b, f32) for b in (bq, bv, bo))
    inst_emb = np.asarray(inst_emb, f32)
    bar_emb = np.asarray(bar_emb, f32)

    def chunked(a):  # (H, X) -> (P, KC_a, X) with row k*P+p -> [p, k]
        kc = a.shape[0] // P
        return np.ascontiguousarray(a.reshape(kc, P, -1).transpose(1, 0, 2))

    # embeddings folded into the context on the host (input prep)
    ctx_e = context + inst_emb[inst] + bar_emb[bars]  # (B, N, H)
    bq_eff = bq + inst_emb[cur] @ Wq.T  # (H,)
    bo_eff = bo + bv @ Wo.T  # V-bias passes through softmax
    WqT = Wq.T.astype(f16)
    WkT = Wk.T.astype(f16)
    WvT = Wv.T.astype(f16)
    WoT = Wo.T.astype(f16)

    in_maps = []
    for b in range(B):
        qT = chunked(query[b].T.astype(f16))
        ctxT = chunked(ctx_e[b].T.astype(f16))
        mbv = np.where(inst[b] == cur, 0.0, 1.0).astype(f32)
        mbt = np.ascontiguousarray(mbv.reshape(NT, P).T)  # (128, NT)
        for g in range(HG):
            sl = slice(g * CH, (g + 1) * CH)
            in_maps.append({
                "qT": qT,
                "ctxT": ctxT,
                "wqT": chunked(WqT[:, sl]),
                "wkT": chunked(WkT[:, sl]),
                "wvT": chunked(WvT[:, sl]),
                "woT": chunked(WoT[sl, :]),
                "mb": mbt,
                "bqe": np.ascontiguousarray(bq_eff[sl].reshape(PT_CH, P).T),
            })
    return in_maps, bo_eff


def kernel(**inputs) -> np.ndarray:
    global _compiled
    if _compiled is None:
        _compiled = _build()
    in_maps, bo_eff = _prep_inputs(**inputs)
    res = run_bass_kernel_spmd(_compiled, in_maps, list(range(B * HG))).results
    out = np.empty((B, T, H), np.float32)
    for b in range(B):
        out[b] = res[b * HG]["out"] + res[b * HG + 1]["out"] + bo_eff
    return out


# revision 21
# speedup vs baseline: 1.1621x; 1.1621x over previous
"""BarCachedCrossAttention Trainium2 kernel.

Sharding: 8 cores = 4 batches x 2 head-groups (8 heads / 512 channels each).
Per core, everything is computed in a transposed layout (partition = context
token for scores) so probs never need a transpose: U^T = V'^T @ P^T with a
ones-column in V' producing the softmax denominators for free.  The
instrument mask is applied by zeroing masked tokens' V' rows (including the
ones-column), so exp needs no per-token bias and batches into 1024-wide ACT
ops.  The K/V projection and the attention (scores -> exp -> U accumulation)
are fused per context slab so ACT exp overlaps projection matmuls.

Host-side folds (pure input prep, the heavy GEMMs all run on device):
  - instrument/bar embeddings are added into the context once on the host
    (the same gather the reference does), so no combo-table / one-hot
    correction matmuls are needed on device;
  - K-bias is dropped (exactly cancels in softmax over n);
  - Q-bias + current instrument embedding fold into a per-channel bias
    applied by the Q-projection's PSUM->SBUF activation;
  - V-bias passes through softmax unchanged (weights sum to 1) and folds
    with the output bias: bo_eff = bo + bv @ Wo.T.

fp16 operands everywhere (1 cyc/row matmuls, FWL weight loads, half DMA);
PSUM/U stay f32.  exp uses a constant -12 shift (cancels in U/Z) keeping the
probs near unity.  Probs and V' are bf16 (range-safe under exp).  Score
matmul head-pairs share one PSUM tile so both 64-row groups become ready
together and run concurrently on the PE (row-group tiling).  1/Z runs at
bf16 for range (Z spans e^-9..2e6).
"""

import sys

sys.path.insert(0, "/opt/trn_rl_repo")

import numpy as np

import concourse.bacc as bacc
import concourse.tile as tile
from concourse import mybir
from concourse.bass_utils import run_bass_kernel_spmd

B, T, N_CTX, H = 4, 512, 2048, 1024
NUM_HEADS, NUM_INSTRUMENTS, MAX_BARS = 16, 16, 8
HEAD_DIM = H // NUM_HEADS  # 64
HG = 2  # head groups (cores per batch)
CH = H // HG  # 512 channels per core
NH_G = NUM_HEADS // HG  # 8 heads per core
P = 128
F32 = mybir.dt.float32
BF16 = mybir.dt.bfloat16
DT = mybir.dt.float16
SHIFT = -12.0  # exp shift keeps bf16 probs in a comfortable range

KC = H // P  # 8 contraction chunks for projections
PT_CH = CH // P  # 4 partition tiles of channels
NT = N_CTX // P  # 16 context tiles of 128 tokens
TT = T // P  # 4 tiles of query tokens
SLABS = [512, 512, 512, 256, 256]  # context slab sizes (sum = N_CTX)

_compiled = None


def _build():
    nc = bacc.Bacc("TRN2", target_bir_lowering=False, debug=False, num_devices=8)

    qT_d = nc.dram_tensor("qT", [P, KC, T], DT, kind="ExternalInput")
    ctxT_d = nc.dram_tensor("ctxT", [P, KC, N_CTX], DT, kind="ExternalInput")
    wq_d = nc.dram_tensor("wqT", [P, KC, CH], DT, kind="ExternalInput")
    wk_d = nc.dram_tensor("wkT", [P, KC, CH], DT, kind="ExternalInput")
    wv_d = nc.dram_tensor("wvT", [P, KC, CH], DT, kind="ExternalInput")
    wo_d = nc.dram_tensor("woT", [P, PT_CH, H], DT, kind="ExternalInput")
    mb_d = nc.dram_tensor("mb", [P, NT], F32, kind="ExternalInput")
    bqe_d = nc.dram_tensor("bqe", [P, PT_CH], F32, kind="ExternalInput")
    out_d = nc.dram_tensor("out", [T, H], F32, kind="ExternalOutput")

    with tile.TileContext(nc) as tc:
        with (
            nc.allow_low_precision(reason="fp16 matmul operands; accum stays f32"),
            tc.tile_pool(name="persist", bufs=1) as pers,
        ):
            # ---- input DMA in need order ----
            # sync: qt/wq chunk pairs (Q-proj streams behind them), small
            # tiles, then wk chunks (K-proj).  gpsimd: slab0 then slabs.
            # scalar (idle until first exp): wv, wo.
            qt = pers.tile([P, KC, T], DT, name="qt_in")
            wq = pers.tile([P, KC, CH], DT, name="wq")
            for k2 in range(0, KC, 2):
                nc.sync.dma_start(qt[:, k2 : k2 + 2, :], qT_d.ap()[:, k2 : k2 + 2, :])
                nc.sync.dma_start(wq[:, k2 : k2 + 2, :], wq_d.ap()[:, k2 : k2 + 2, :])
            mb = pers.tile([P, NT], F32, name="mb")
            nc.sync.dma_start(mb[:], mb_d.ap())
            bqe = pers.tile([P, PT_CH], F32, name="bqe")
            nc.sync.dma_start(bqe[:], bqe_d.ap())
            wk = pers.tile([P, KC, CH], DT, name="wk")
            wv = pers.tile([P, KC, CH], DT, name="wv")
            wo = pers.tile([P, PT_CH, H], DT, name="wo")

            ones1v = pers.tile([1, P], F32, name="ones1v")
            nc.vector.memset(ones1v[:], 1.0)
            ones8 = pers.tile([P, NH_G], F32, name="ones8")
            nc.vector.memset(ones8[:], 1.0)
            shiftb = pers.tile([P, 1], F32, name="shiftb")
            nc.vector.memset(shiftb[:], SHIFT)

            QT = [pers.tile([P, T], DT, name=f"qt{p}") for p in range(PT_CH)]
            OT = [pers.tile([P, T], DT, name=f"ot{p}") for p in range(PT_CH)]
            U = [pers.tile([HEAD_DIM + 1, T], F32, name=f"u{h}") for h in range(NH_G)]
            ZS = [pers.tile([1, 2, 512], F32, name=f"zs{hp}") for hp in range(NH_G // 2)]
            RF = [pers.tile([1, 2, 512], F32, name=f"rf{hp}") for hp in range(NH_G // 2)]

            # ---- Q projection (k-major: streams behind the chunked DMA) ----
            with tc.tile_pool(name="qps", bufs=1, space="PSUM") as qps:
                ps_q = [qps.tile([P, 512], F32, name=f"ps_q{p}") for p in range(PT_CH)]
                for k in range(KC):
                    for p in range(PT_CH):
                        nc.tensor.matmul(
                            ps_q[p][:],
                            wq[:, k, p * P : (p + 1) * P],
                            qt[:, k, :],
                            start=(k == 0),
                            stop=(k == KC - 1),
                        )
                for p in range(PT_CH):
                    nc.scalar.activation(
                        QT[p][:], ps_q[p][:], mybir.ActivationFunctionType.Identity,
                        bias=bqe[:, p : p + 1], scale=1.0,
                    )

            # ---- fused K/V projection + attention, one context slab at a time ----
            NS = len(SLABS)
            offs = [sum(SLABS[:i]) for i in range(NS)]

            def emit_proj(si, kvsb, kvps, preloaded=None):
                """K^T and V' tiles for slab si; returns (kts, vts)."""
                n0, sl = offs[si], SLABS[si]
                s4n = sl // P
                if preloaded is None:
                    slab = slabp.tile([P, KC, 512], DT, name="slab")
                    nc.sync.dma_start(
                        slab[:, :, :sl], ctxT_d.ap()[:, :, n0 : n0 + sl]
                    )
                else:
                    slab = preloaded
                kts = []
                for p in range(PT_CH):
                    ps = kvps.tile([P, 512], F32, name="ps_kv")
                    for k in range(KC):
                        nc.tensor.matmul(
                            ps[:, :sl],
                            wk[:, k, p * P : (p + 1) * P],
                            slab[:, k, :sl],
                            start=(k == 0), stop=(k == KC - 1),
                        )
                    kt = kvsb.tile([P, 512], DT, name=f"kt{p}")
                    nc.vector.tensor_copy(kt[:, :sl], ps[:, :sl])
                    kts.append(kt)
                vts = []
                for s4 in range(s4n):
                    i = (n0 // P) + s4
                    psv = kvps.tile([P, 512], F32, name="ps_kv")
                    for k in range(KC):
                        nc.tensor.matmul(
                            psv[:],
                            slab[:, k, s4 * P : (s4 + 1) * P],
                            wv[:, k, :],
                            start=(k == 0), stop=(k == KC - 1),
                        )
                    vt = kvsb.tile([P, NH_G, HEAD_DIM + 1], BF16, name=f"v{s4}")
                    nc.vector.tensor_scalar_mul(
                        vt[:, :, :HEAD_DIM],
                        psv[:].rearrange("p (h d) -> p h d", d=HEAD_DIM),
                        mb[:, i : i + 1],
                    )
                    nc.vector.tensor_scalar_mul(
                        vt[:, :, HEAD_DIM], ones8[:], mb[:, i : i + 1]
                    )
                    vts.append(vt)
                return kts, vts

            with (
                tc.tile_pool(name="slab", bufs=2) as slabp,
                tc.tile_pool(name="kvsb", bufs=2) as kvsb,
                tc.tile_pool(name="ptp", bufs=4) as ptp,
                tc.tile_pool(name="kvps", bufs=2, space="PSUM") as kvps,
                tc.tile_pool(name="sps", bufs=2, space="PSUM") as sps,
                tc.tile_pool(name="ups", bufs=1, space="PSUM") as ups,
            ):
                def emit_norm(hp):
                    for hi in range(2):
                        h = 2 * hp + hi
                        psr = kvps.tile([P, 512], F32, name="ps_kv")
                        nc.tensor.matmul(
                            psr[:HEAD_DIM, :], ones1v[:, :HEAD_DIM],
                            RF[hp][:, hi, :], start=True, stop=True,
                        )
                        nc.vector.tensor_tensor(
                            OT[hp][hi * HEAD_DIM : (hi + 1) * HEAD_DIM, :],
                            U[h][:HEAD_DIM, :],
                            psr[:HEAD_DIM, :],
                            op=mybir.AluOpType.mult,
                        )

                # slab0 + wk interleaved chunk pairs, then wv chunks, then
                # wo -- all on the sync queue so the DMA engines see
                # descriptors in exact need order (big early transfers on
                # other queues would starve the critical path).
                first_slab = slabp.tile([P, KC, 512], DT, name="slab")
                for k2 in range(0, KC, 2):
                    nc.sync.dma_start(
                        first_slab[:, k2 : k2 + 2, :],
                        ctxT_d.ap()[:, k2 : k2 + 2, 0:512],
                    )
                    nc.sync.dma_start(
                        wk[:, k2 : k2 + 2, :], wk_d.ap()[:, k2 : k2 + 2, :]
                    )
                for k2 in range(0, KC, 2):
                    nc.sync.dma_start(
                        wv[:, k2 : k2 + 2, :], wv_d.ap()[:, k2 : k2 + 2, :]
                    )
                nc.sync.dma_start(wo[:], wo_d.ap())
                cur = emit_proj(0, kvsb, kvps, preloaded=first_slab)
                for ns in range(NS):
                    kts, vts = cur
                    s4n = SLABS[ns] // P
                    # attention: per (head pair, 128-token ctx chunk): the
                    # score pair shares one PSUM tile -> both row groups
                    # become ready together and run concurrently on the PE.
                    for hp in range(NH_G // 2):
                        psus = [
                            ups.tile([HEAD_DIM + 1, 512], F32, name=f"ps_u{hi}")
                            for hi in range(2)
                        ]
                        for s4 in range(s4n):
                            pss = sps.tile([P, 2, 512], F32, name="ps_s")
                            pts = ptp.tile([P, 2, 512], BF16, name="pt")
                            for hi in range(2):
                                d0, d1 = hi * HEAD_DIM, (hi + 1) * HEAD_DIM
                                nc.tensor.matmul(
                                    pss[:, hi, :],
                                    kts[hp][d0:d1, s4 * P : (s4 + 1) * P],
                                    QT[hp][d0:d1, :],
                                    start=True, stop=True,
                                )
                            nc.scalar.activation(
                                pts[:], pss[:], mybir.ActivationFunctionType.Exp,
                                bias=shiftb[:], scale=0.125,
                            )
                            for hi in range(2):
                                nc.tensor.matmul(
                                    psus[hi][:],
                                    vts[s4][:, 2 * hp + hi, :],
                                    pts[:, hi, :],
                                    start=(s4 == 0), stop=(s4 == s4n - 1),
                                )
                        for hi in range(2):
                            h = 2 * hp + hi
                            if ns == 0:
                                nc.vector.tensor_copy(U[h][:], psus[hi][:])
                            else:
                                nc.vector.tensor_add(U[h][:], U[h][:], psus[hi][:])
                        if ns == NS - 1:
                            # normalization, software-pipelined one head pair
                            # behind the attention: the DVE reciprocal chain
                            # for hp runs under hp+1's attention, and the psr
                            # broadcast + OT multiply for hp-1 are emitted
                            # here so OT is written well before the O
                            # projection reads it.
                            for hi in range(2):
                                h = 2 * hp + hi
                                nc.scalar.copy(
                                    ZS[hp][:, hi, :],
                                    U[h][HEAD_DIM : HEAD_DIM + 1, :],
                                )
                            nc.vector.reciprocal_approx_fast(RF[hp][:], ZS[hp][:])
                            if hp > 0:
                                emit_norm(hp - 1)
                    if ns + 1 < NS:
                        cur = emit_proj(ns + 1, kvsb, kvps)
                    else:
                        emit_norm(NH_G // 2 - 1)

            # ---- normalization broadcast + output projection ----
            with (
                tc.tile_pool(name="ob", bufs=3) as obp,
                tc.tile_pool(name="ops", bufs=2, space="PSUM") as ops,
            ):
                # O = OT.T @ WoT (partial over this head-group's channels)
                for tt in range(TT):
                    for o in range(2):
                        pso = ops.tile([P, 512], F32, name="ps_o")
                        for p in range(PT_CH):
                            nc.tensor.matmul(
                                pso[:],
                                OT[p][:, tt * P : (tt + 1) * P],
                                wo[:, p, o * 512 : (o + 1) * 512],
                                start=(p == 0), stop=(p == PT_CH - 1),
                            )
                        ob = obp.tile([P, 512], F32, name="ob")
                        if o == 0:
                            nc.vector.tensor_copy(ob[:], pso[:])
                        else:
                            nc.scalar.copy(ob[:], pso[:])
                        nc.sync.dma_start(
                            out_d.ap()[tt * P : (tt + 1) * P, o * 512 : (o + 1) * 512],
                            ob[:],
                        )

    nc.compile()
    return nc


def _prep_inputs(query, context, instrument_ids, current_instrument_id, bar_offsets,
                 Wq, bq, Wk, bk, Wv, bv, Wo, bo, inst_emb, bar_emb):
    f32, f16 = np.float32, np.float16
    query = np.asarray(query, f32)
    context = np.asarray(context, f32)
    inst = np.asarray(instrument_ids).astype(np.int64)
    bars = np.clip(np.asarray(bar_offsets).astype(np.int64), 0, MAX_BARS - 1)
    cur = min(max(int(np.asarray(current_instrument_id)), 0), NUM_INSTRUMENTS - 1)
    Wq, Wk, Wv, Wo = (np.asarray(w, f32) for w in (Wq, Wk, Wv, Wo))
    bq, bv, bo = (np.asarray(b, f32) for b in (bq, bv, bo))
    inst_emb = np.asarray(inst_emb, f32)
    bar_emb = np.asarray(bar_emb, f32)

    def chunked(a):  # (H, X) -> (P, KC_a, X) with row k*P+p -> [p, k]
        kc = a.shape[0] // P
        return np.ascontiguousarray(a.reshape(kc, P, -1).transpose(1, 0, 2))

    # embeddings folded into the context on the host (input prep)
    ctx_e = context + inst_emb[inst] + bar_emb[bars]  # (B, N, H)
    bq_eff = bq + inst_emb[cur] @ Wq.T  # (H,)
    bo_eff = bo + bv @ Wo.T  # V-bias passes through softmax
    WqT = Wq.T.astype(f16)
    WkT = Wk.T.astype(f16)
    WvT = Wv.T.astype(f16)
    WoT = Wo.T.astype(f16)

    in_maps = []
    for b in range(B):
        qT = chunked(query[b].T.astype(f16))
        ctxT = chunked(ctx_e[b].T.astype(f16))
        mbv = np.where(inst[b] == cur, 0.0, 1.0).astype(f32)
        mbt = np.ascontiguousarray(mbv.reshape(NT, P).T)  # (128, NT)
        for g in range(HG):
            sl = slice(g * CH, (g + 1) * CH)
            in_maps.append({
                "qT": qT,
                "ctxT": ctxT,
                "wqT": chunked(WqT[:, sl]),
                "wkT": chunked(WkT[:, sl]),
                "wvT": chunked(WvT[:, sl]),
                "woT": chunked(WoT[sl, :]),
                "mb": mbt,
                "bqe": np.ascontiguousarray(bq_eff[sl].reshape(PT_CH, P).T),
            })
    return in_maps, bo_eff


def kernel(**inputs) -> np.ndarray:
    global _compiled
    if _compiled is None:
        _compiled = _build()
    in_maps, bo_eff = _prep_inputs(**inputs)
    res = run_bass_kernel_spmd(_compiled, in_maps, list(range(B * HG))).results
    out = np.empty((B, T, H), np.float32)
    for b in range(B):
        out[b] = res[b * HG]["out"] + res[b * HG + 1]["out"] + bo_eff
    return out
